# revision 15
# baseline (speedup 1.0000x reference)
"""Trainium2 Bass kernel for nn_CausalSelfAttention (B=2, T=2048, C=1024, 16 heads).

Sharding: 8 cores = 2 batches x 4 head-groups (4 heads each). Each core:
  - computes Q^T/K^T ([d,t] layout) and V ([t,d]) for its heads from x^T
    (host pre-transposes x and pre-slices the weights),
  - runs causal flash attention: S^T tiles [k,q] -> exp on ScalarE (scale
    fused, no max-subtraction; logits are O(1)) -> P@V oriented [q, d+1]
    (pt as stationary operand -> full 128 contraction; a ones-column in V
    gives softmax denominators per q-partition) -> per-partition normalize
    (reciprocal + per-head scalar muls) -> O^T via XBAR DMA-transpose,
  - projects through its W_out row-slice producing a partial [T, C] output.
Host sums the 4 tensor-parallel partials per batch (the "all-reduce") and
adds b_out.

The exp stream on ScalarE (58us) is ~2x the S-matmul time, so the psS ring
paces S tiles at Act speed; QKV-projection chains for the next quarter and
the previous block's out-projection are woven between S tiles (per-engine
queues are in-order) to keep PE busy while Act drains.
"""
import sys

if '/opt/trn_rl_repo' not in sys.path:
    sys.path.insert(0, '/opt/trn_rl_repo')

import numpy as np
import ml_dtypes

B, T, C = 2, 2048, 1024
N_HEAD = 16
D = 64
P = 128
N_CORES = 8
GROUPS = N_CORES // B            # 4 tensor-parallel groups per batch
HPC = N_HEAD // GROUPS           # 4 heads per core
DH = HPC * D                     # 256 head dims per core
KO = C // P                      # 8 contraction subtiles for projections
NQB = T // 512                   # 4 q blocks of 512
SCALE = 1.0 / np.sqrt(D)

_CACHE = {}


def _build():
    import concourse.mybir as mybir
    import concourse.tile as tile
    from concourse import bacc

    f32 = mybir.dt.float32
    bf16 = mybir.dt.bfloat16
    f16 = mybir.dt.float16

    nc = bacc.Bacc("TRN2", target_bir_lowering=False, debug=False,
                   num_devices=N_CORES)

    xt_d = nc.dram_tensor("xt", [C, T], bf16, kind="ExternalInput")
    w_d = nc.dram_tensor("w", [C, 3 * DH], bf16, kind="ExternalInput")
    bqkv_d = nc.dram_tensor("bqkv", [3 * DH], f32, kind="ExternalInput")
    wo_d = nc.dram_tensor("wo", [DH, C], bf16, kind="ExternalInput")
    out_d = nc.dram_tensor("out", [T, C], f16, kind="ExternalOutput")

    EXP = mybir.ActivationFunctionType.Exp
    ADD = mybir.AluOpType.add
    pls = [slice(0, D), slice(D, 2 * D)]

    with tile.TileContext(nc) as tc:
        with tc.tile_pool(name="persist", bufs=1) as pp:
            xts = [pp.tile([P, KO, 512], bf16, tag=f"xt{q}", name=f"xt{q}")
                   for q in range(4)]
            wqk = pp.tile([P, KO, 2 * DH], bf16, tag="wqk")
            wv = pp.tile([P, KO, DH], bf16, tag="wv")
            wo = pp.tile([P, 2, C], bf16, tag="wo")
            bq = pp.tile([P, 2], f32, tag="bq")
            bk = pp.tile([P, 2], f32, tag="bk")
            bvrow = pp.tile([1, DH], f32, tag="bvrow")
            bvb = pp.tile([P, DH], f32, tag="bvb")
            qts = [[pp.tile([P, 512], bf16, tag=f"qt{s}_{q}", name=f"qt{s}_{q}")
                    for q in range(4)] for s in range(2)]
            kts = [[pp.tile([P, 512], bf16, tag=f"kt{s}_{q}", name=f"kt{s}_{q}")
                    for q in range(4)] for s in range(2)]
            vos = [pp.tile([P, 4, HPC, D + 1], bf16, tag=f"vo{q}",
                           name=f"vo{q}") for q in range(4)]
            ots = [[pp.tile([P, 512], bf16, tag=f"ot{j}_{s}",
                            name=f"ot{j}_{s}") for s in range(2)]
                   for j in range(NQB)]
            scr = pp.tile([1, 1], f32, tag="scr")

            # ---- input DMAs: 2-ko chunks of wqk/xt-quarter-0 interleaved so
            # the first projection matmul starts ~3us in; rest in need order.
            xt_r = xt_d.rearrange("(ko p) t -> p ko t", p=P)
            w_r = w_d.rearrange("(ko p) n -> p ko n", p=P)
            for c2 in range(4):
                ks = slice(2 * c2, 2 * c2 + 2)
                nc.sync.dma_start(wqk[:, ks, :], w_r[:, ks, 0:2 * DH])
                nc.sync.dma_start(xts[0][:, ks, :], xt_r[:, ks, 0:512])
            nc.sync.dma_start(bq[:], bqkv_d[0:DH].rearrange("(s p) -> p s", p=P))
            nc.sync.dma_start(bk[:], bqkv_d[DH:2 * DH].rearrange("(s p) -> p s", p=P))
            nc.sync.dma_start(bvrow[0:1, :],
                              bqkv_d[2 * DH:3 * DH].rearrange("(o n) -> o n", o=1))
            nc.sync.dma_start(wv[:], w_r[:, :, 2 * DH:3 * DH])
            nc.sync.dma_start(xts[1][:], xt_r[:, :, 512:1024])
            nc.sync.dma_start(xts[2][:], xt_r[:, :, 1024:1536])
            nc.sync.dma_start(wo[:], wo_d.rearrange("(s p) c -> p s c", p=P))
            nc.sync.dma_start(xts[3][:], xt_r[:, :, 1536:2048])

            # ---- prologue constants ----
            nc.gpsimd.partition_broadcast(bvb[:, :], bvrow[0:1, :])
            for q in range(4):
                nc.vector.memset(vos[q][:, :, :, D:D + 1], 1.0)
            iden = pp.tile([P, P], f32, tag="iden")
            nc.gpsimd.memset(iden[:], 1.0)
            nc.gpsimd.affine_select(
                out=iden[:], in_=iden[:],
                compare_op=mybir.AluOpType.is_equal, fill=0.0,
                base=0, pattern=[[1, P]], channel_multiplier=-1)
            # trigger the exp ACT-table load early
            nc.vector.memset(scr[0:1, 0:1], 0.0)
            nc.scalar.activation(scr[0:1, 0:1], scr[0:1, 0:1], EXP)

            pts = {}

            with (
                tc.tile_pool(name="psS", bufs=2, space="PSUM") as psS,
                tc.tile_pool(name="psB", bufs=4, space="PSUM") as psB,
                tc.tile_pool(name="ptp", bufs=34) as ptp,
                tc.tile_pool(name="wrk", bufs=4) as wrk,
                tc.tile_pool(name="obp", bufs=3) as obp,
            ):
                # --- filler items: projection chains + out-projection ---
                _acc = {}

                def item_qk(q, s_, which, koh):
                    # one chain per 1-bank acc tile, emitted as two ko-halves
                    # so S tiles can weave between (keeps the Act exp queue
                    # fed); the PSUM group stays open across the gap
                    key = ("qk", q, s_, which)
                    if koh == 0:
                        _acc[key] = psB.tile([P, 4, 128], f32, tag="acc",
                                             name=f"pq{q}_{s_}_{which}")
                    sp = _acc[key].rearrange("p a b -> p (a b)")
                    wofs = s_ * P if which == 0 else DH + s_ * P
                    for ko in range(4 * koh, 4 * koh + 4):
                        nc.tensor.matmul(
                            sp[:, :], wqk[:, ko, wofs:wofs + P],
                            xts[q][:, ko, :],
                            start=(ko == 0), stop=(ko == KO - 1))
                    if koh == 1:
                        dst, bias = ((qts[s_][q], bq) if which == 0
                                     else (kts[s_][q], bk))
                        nc.vector.tensor_scalar_add(
                            dst[:, :], sp[:, :], bias[:, s_:s_ + 1])

                def item_v(q, u2, koh):
                    i = 4 * q + u2
                    key = ("v", q, u2)
                    if koh == 0:
                        _acc[key] = psB.tile([P, 4, 128], f32, tag="acc",
                                             name=f"pv{q}_{u2}")
                    sv = _acc[key].rearrange("p a b -> p (a b)")
                    for ko in range(4 * koh, 4 * koh + 4):
                        nc.tensor.matmul(
                            sv[:, 0:DH],
                            xts[q][:, ko, (i % 4) * P:(i % 4 + 1) * P],
                            wv[:, ko, :],
                            start=(ko == 0), stop=(ko == KO - 1))
                    if koh == 1:
                        nc.vector.tensor_tensor(
                            vos[q][:, i % 4, :, 0:D],
                            sv[:, 0:DH].rearrange("p (h d) -> p h d", h=HPC),
                            bvb.rearrange("p (h d) -> p h d", h=HPC),
                            ADD)

                def item_outproj(j, mo, n, ob_act=False):
                    m = 4 * j + mo
                    acc = psB.tile([P, 4, 128], f32, tag="acc",
                                   name=f"pc{j}_{mo}_{n}")
                    pc = acc.rearrange("p a b -> p (a b)")
                    for s in range(2):
                        nc.tensor.matmul(
                            pc[:, :],
                            ots[j][s][:, mo * P:(mo + 1) * P],
                            wo[:, s, n * 512:(n + 1) * 512],
                            start=(s == 0), stop=(s == 1))
                    ob = obp.tile([P, 512], f16, tag="ob")
                    if ob_act:
                        nc.scalar.copy(ob[:], pc[:, :])
                    else:
                        nc.vector.tensor_copy(ob[:], pc[:, :])
                    nc.sync.dma_start(
                        out_d[m * P:(m + 1) * P, n * 512:(n + 1) * 512],
                        ob[:])

                def proj_items(q):
                    its = []
                    for s_ in range(2):
                        for w in range(2):
                            for koh in range(2):
                                its.append(lambda s_=s_, w=w, koh=koh:
                                           item_qk(q, s_, w, koh))
                    for u2 in range(4):
                        for koh in range(2):
                            its.append(lambda u2=u2, koh=koh:
                                       item_v(q, u2, koh))
                    return its

                def outproj_items(j):
                    return [(lambda mo=mo, n=n: item_outproj(j, mo, n))
                            for mo in range(4) for n in range(2)]

                # --- attention per-block emitters ---
                def item_S(j, i):
                    off = max(0, P * i - 512 * j)
                    width = 512 - off
                    for hs in range(2):
                        sp = psS.tile([P, 2, 512], f32, tag="sp",
                                      name=f"sp{j}_{hs}_{i}")
                        for u in range(2):
                            nc.tensor.matmul(
                                sp[:, u, 0:width],
                                kts[hs][i // 4][pls[u],
                                                (i % 4) * P:(i % 4 + 1) * P],
                                qts[hs][j][pls[u], off:512],
                                start=True, stop=True)
                        pt = ptp.tile([P, 2, 512], bf16, tag="pt",
                                      name=f"pt{j}_{hs}_{i}")
                        nc.scalar.activation(pt[:, :, 0:width],
                                             sp[:, :, 0:width],
                                             EXP, scale=float(SCALE))
                        if i >= 4 * j:  # diagonal: zero the q < k triangle
                            for u in range(2):
                                nc.gpsimd.affine_select(
                                    out=pt[:, u, 0:P], in_=pt[:, u, 0:P],
                                    compare_op=mybir.AluOpType.is_ge,
                                    fill=0.0, base=0, pattern=[[1, P]],
                                    channel_multiplier=-1)
                        pts[(j, hs, i)] = pt

                def item_PV(j, qs):
                    ilast = 4 * j + qs
                    po = psB.tile([P, 4, 128], f32, tag="acc",
                                  name=f"po{j}_{qs}")
                    for hs in range(2):
                        for u in range(2):
                            h = 2 * hs + u
                            for i in range(ilast + 1):
                                a = qs * P - max(0, P * i - 512 * j)
                                nc.tensor.matmul(
                                    po[:, h, 0:D + 1],
                                    pts[(j, hs, i)][:, u, a:a + P],
                                    vos[i // 4][:, i % 4, h, :],
                                    start=(h == 0 and i == 0),
                                    stop=(h == 3 and i == ilast))
                    rcp = wrk.tile([P, 4, 1], f32, tag="rcp")
                    nc.vector.reciprocal_approx_fast(rcp[:], po[:, :, D:D + 1])
                    if j < NQB - 1:
                        # normalize on Act (idle between exp streams), then
                        # O^T via XBAR DMA-transpose; out-proj consumes it a
                        # block later so the ~3.5us DMA latency is hidden
                        osb = wrk.tile([P, 4, D], bf16, tag="osb")
                        for h in range(4):
                            nc.scalar.mul(
                                osb[:, h, :], po[:, h, 0:D], rcp[:, h, 0:1])
                        for s in range(2):
                            nc.sync.dma_start_transpose(
                                ots[j][s][:, qs * P:(qs + 1) * P],
                                osb[:, 2 * s:2 * s + 2, :])
                    else:
                        osb = wrk.tile([P, 4, D], bf16, tag="osb32")
                        for h in range(4):
                            nc.scalar.mul(
                                osb[:, h, :], po[:, h, 0:D], rcp[:, h, 0:1])
                        for s in range(2):
                            nc.sync.dma_start_transpose(
                                ots[j][s][:, qs * P:(qs + 1) * P],
                                osb[:, 2 * s:2 * s + 2, :])

                def emit_block(j, fillers):
                    """Weave filler items between S tiles (PE queue is
                    in-order; S tiles are paced by the Act exp stream via the
                    psS ring), then PV with remaining fillers interleaved."""
                    nkt = 4 * (j + 1)
                    work = [(lambda i=i: item_S(j, i)) for i in range(nkt)]
                    nf = len(fillers)
                    head = fillers
                    # proportional merge keeping both orders (work leads)
                    iw = fi = 0
                    while iw < len(work) or fi < len(head):
                        if fi >= len(head) or (iw < len(work)
                                               and iw * len(head) <= fi * len(work)):
                            work[iw]()
                            iw += 1
                        else:
                            head[fi]()
                            fi += 1
                    for qs in range(4):
                        item_PV(j, qs)

                # ---- schedule ----
                for s_ in range(2):
                    for w in range(2):
                        for koh in range(2):
                            item_qk(0, s_, w, koh)
                for u2 in range(4):
                    for koh in range(2):
                        item_v(0, u2, koh)
                emit_block(0, proj_items(1))
                emit_block(1, proj_items(2))
                emit_block(2, proj_items(3) + outproj_items(0))
                emit_block(3, outproj_items(1) + outproj_items(2))
                for mo in range(4):
                    for n in range(2):
                        item_outproj(3, mo, n, ob_act=True)

    nc.compile()
    return nc


def _get_nc():
    if "nc" not in _CACHE:
        _CACHE["nc"] = _build()
    return _CACHE["nc"]


def _get_runner():
    """Build the jitted SPMD executor once (mirrors bass2jax.run_bass_via_pjrt
    but caches the jitted function so repeat calls skip retrace/recompile)."""
    if "runner" in _CACHE:
        return _CACHE["runner"]
    import jax
    import numpy as _np
    from jax.sharding import Mesh, PartitionSpec
    from jax.experimental.shard_map import shard_map
    import concourse.mybir as mybir
    from concourse import bass2jax

    nc = _get_nc()
    bass2jax.install_neuronx_cc_hook()

    partition_name = (nc.partition_id_tensor.name
                      if nc.partition_id_tensor else None)
    in_names, out_names, out_avals, zero_shapes = [], [], [], []
    for alloc in nc.m.functions[0].allocations:
        if not isinstance(alloc, mybir.MemoryLocationSet):
            continue
        name = alloc.memorylocations[0].name
        if alloc.kind == "ExternalInput":
            if name != partition_name:
                in_names.append(name)
        elif alloc.kind == "ExternalOutput":
            out_avals.append(jax.core.ShapedArray(
                tuple(alloc.tensor_shape), mybir.dt.np(alloc.dtype)))
            out_names.append(name)
            zero_shapes.append((tuple(alloc.tensor_shape),
                                mybir.dt.np(alloc.dtype)))
    n_params = len(in_names)
    n_outs = len(out_names)
    all_names = in_names + out_names
    if partition_name is not None:
        all_names = all_names + [partition_name]

    def _body(*args):
        operands = list(args)
        if partition_name is not None:
            operands.append(bass2jax.partition_id_tensor())
        outs = bass2jax._bass_exec_p.bind(
            *operands,
            out_avals=tuple(out_avals),
            in_names=tuple(all_names),
            out_names=tuple(out_names),
            lowering_input_output_aliases=(),
            sim_require_finite=True,
            sim_require_nnan=True,
            nc=nc,
        )
        return tuple(outs)

    devices = jax.devices()[:N_CORES]
    mesh = Mesh(_np.asarray(devices), ("core",))
    donate = tuple(range(n_params, n_params + n_outs))
    sharded = jax.jit(
        shard_map(_body, mesh=mesh,
                  in_specs=(PartitionSpec("core"),) * (n_params + n_outs),
                  out_specs=(PartitionSpec("core"),) * n_outs,
                  check_rep=False),
        donate_argnums=donate, keep_unused=True)

    def run(in_maps):
        concat_in = [
            _np.concatenate([_np.asarray(m[name]) for m in in_maps], axis=0)
            for name in in_names]
        concat_zeros = [
            _np.zeros((N_CORES * sh[0], *sh[1:]), dtype)
            for sh, dtype in zero_shapes]
        out_arrs = sharded(*concat_in, *concat_zeros)
        return [
            {name: _np.asarray(out_arrs[i]).reshape(
                N_CORES, *zero_shapes[i][0])[c]
             for i, name in enumerate(out_names)}
            for c in range(N_CORES)]

    _CACHE["runner"] = run
    return run


def kernel(x, mask, W_qkv, b_qkv, W_out, b_out):

    bf = ml_dtypes.bfloat16
    x = np.asarray(x, dtype=np.float32)
    W_qkv = np.asarray(W_qkv, dtype=np.float32)
    b_qkv = np.asarray(b_qkv, dtype=np.float32)
    W_out = np.asarray(W_out, dtype=np.float32)
    b_out = np.asarray(b_out, dtype=np.float32)
    # mask is the causal tril mask (per problem spec); causality is
    # implemented structurally on-device.

    run = _get_runner()

    xts = [np.ascontiguousarray(x[b].T).astype(bf) for b in range(B)]
    in_maps = []
    for core in range(N_CORES):
        b, g = divmod(core, GROUPS)
        cs = slice(g * DH, (g + 1) * DH)
        w_c = np.concatenate(
            [W_qkv[:, cs], W_qkv[:, C:][:, cs], W_qkv[:, 2 * C:][:, cs]],
            axis=1).astype(bf)
        bq_c = np.concatenate(
            [b_qkv[cs], b_qkv[C:][cs], b_qkv[2 * C:][cs]]).astype(np.float32)
        wo_c = np.ascontiguousarray(W_out[cs, :]).astype(bf)
        in_maps.append({"xt": xts[b], "w": np.ascontiguousarray(w_c),
                        "bqkv": bq_c, "wo": wo_c})

    results = run(in_maps)

    out = np.zeros((B, T, C), dtype=np.float32)
    for core in range(N_CORES):
        b = core // GROUPS
        out[b] += results[core]["out"].astype(np.float32)
    out += b_out[None, None, :]
    return out
